# revision 32
# baseline (speedup 1.0000x reference)
"""Trainium2 Bass kernel for nn_Encoding_layer (highway stack + pairwise MLP
attention + fuse gates).

Sharding: data-parallel over batch B=16 across 8 NeuronCores (2 batches per
core); all dense weights replicated. No collectives.

Per-core layouts (n = 2 batches x L=1024 = 2048 token-columns):
  xTh/x1T/x2T/w3x/attT : [128, 4, 2048] bf16, "transposed" activations
                         [u mod 128, u div 128, n]
  xO                   : [128, 16, 512] bf16, row-major highway output
                         [row mod 128, row div 128, u]
  Attention: S^T[j,i] = s3[j,i] (PE, w3*x^T as lhsT) + s2[j] (ACT exp bias).
  The per-column term s1[i]+ab never enters the matmuls: exp(S+s1+ab) =
  exp(s1+ab)*exp(S), and a per-column factor cancels in the softmax, so
  relu becomes a clamp against th[i] = exp(-(s1[i]+ab)):
      M^T = max(exp(s3+s2), th)  ==  exp(relu(S_full)) / exp(s1+ab)
  Numerator att^T (lhsT = row-major x) and denominator r (lhsT = ones
  column) come from matmuls against M^T; normalization multiplies by the
  broadcast fast-approx reciprocal of r.  Broadcasts of [1,512] rows are
  PE outer-products (ones_row as lhsT) + scalar-engine copies - gpsimd
  partition_broadcast triggers multi-us ucode LIBRARY_RELOAD stalls.
"""

import numpy as np

B, L, U, H = 16, 1024, 512, 2
NCORES = 8
BPC = B // NCORES          # batches per core
N = BPC * L                # token columns per core
KU = U // 128              # 4  u-tiles
NT = N // 128              # 16 row-tiles per core
NS = N // 512              # 4  512-wide column slices per core
JT = L // 128              # 8  j-tiles per batch
IH = L // 512              # 2  i-halves per batch


def build_nc():
    import concourse.bacc as bacc
    import concourse.tile as tile
    from concourse import mybir
    from concourse.masks import make_identity

    F32 = mybir.dt.float32
    BF16 = mybir.dt.bfloat16
    AF = mybir.ActivationFunctionType
    OP = mybir.AluOpType

    nc = bacc.Bacc("TRN2", target_bir_lowering=False, debug=False,
                   num_devices=NCORES)

    x_in = nc.dram_tensor("inputs", [BPC, L, U], F32, kind="ExternalInput").ap()
    tW = nc.dram_tensor("tW", [H, U, U], F32, kind="ExternalInput").ap()
    tb = nc.dram_tensor("tb", [H, U], F32, kind="ExternalInput").ap()
    cW = nc.dram_tensor("cW", [H, U, U], F32, kind="ExternalInput").ap()
    cb = nc.dram_tensor("cb", [H, U], F32, kind="ExternalInput").ap()
    aW = nc.dram_tensor("aW", [3 * U], F32, kind="ExternalInput").ap()
    ab = nc.dram_tensor("ab", [1], F32, kind="ExternalInput").ap()
    frW = nc.dram_tensor("frW", [2 * U, U], F32, kind="ExternalInput").ap()
    frb = nc.dram_tensor("frb", [U], F32, kind="ExternalInput").ap()
    ffW = nc.dram_tensor("ffW", [2 * U, U], F32, kind="ExternalInput").ap()
    ffb = nc.dram_tensor("ffb", [U], F32, kind="ExternalInput").ap()
    out = nc.dram_tensor("out", [BPC, L, U], F32, kind="ExternalOutput").ap()

    xv = x_in.flatten_outer_dims().rearrange("(t p) u -> t p u", p=128)
    outv = out.flatten_outer_dims().rearrange("(t p) u -> t p u", p=128)

    with tile.TileContext(nc) as tc:
        with tc.tile_pool(name="pers", bufs=1) as pers:
            # ---- persistent SBUF tensors ----
            xTh = pers.tile([128, KU, N], BF16, tag="xTh")    # inputs^T
            x1T = pers.tile([128, KU, N], BF16, tag="x1T")
            x2T = pers.tile([128, KU, N], BF16, tag="x2T")
            w3x = pers.tile([128, KU, N], BF16, tag="w3x")
            attT = pers.tile([128, KU, N], BF16, tag="attT")
            xO = pers.tile([128, NT, U], BF16, tag="xO")
            tWh = pers.tile([128, H, KU, U], BF16, tag="tWh")
            cWh = pers.tile([128, H, KU, U], BF16, tag="cWh")
            ffWh = pers.tile([128, 2 * KU, U], BF16, tag="ffWh")
            frWh = pers.tile([128, 2 * KU, U], BF16, tag="frWh")
            tbsb = pers.tile([128, H, KU], F32, tag="tbsb")
            cbsb = pers.tile([128, H, KU], F32, tag="cbsb")
            awsb = pers.tile([128, 12], F32, tag="awsb")      # w1|w2|w3 cols
            w1h = pers.tile([128, KU], BF16, tag="w1h")
            w2h = pers.tile([128, KU], BF16, tag="w2h")
            ab_sb = pers.tile([1, 1], F32, tag="ab_sb")
            nab_sb = pers.tile([1, 1], F32, tag="nab_sb")
            ffb_h = pers.tile([1, U], BF16, tag="ffb_h")
            frb_h = pers.tile([1, U], BF16, tag="frb_h")
            thr = pers.tile([1, N], BF16, tag="thr")   # exp(-(s1+ab))
            s2f = pers.tile([128, NT], F32, tag="s2f")
            ones_row = pers.tile([1, 128], BF16, tag="ones_row")
            ones_col = pers.tile([128, 1], BF16, tag="ones_col")
            ident = pers.tile([128, 128], BF16, tag="ident")
            identf = pers.tile([128, 128], F32, tag="identf")

            nc.vector.memset(ones_row, 1.0)
            nc.vector.memset(ones_col, 1.0)
            make_identity(nc, ident)
            make_identity(nc, identf)

            # ================= Phase A: loads, casts, input transpose ======
            with tc.tile_pool(name="stg", bufs=8) as stg, \
                 tc.tile_pool(name="stgw", bufs=8) as stgw, \
                 tc.tile_pool(name="ptA", bufs=1, space="PSUM") as ptA:
                # inputs^T via PE transpose (PE is idle here), with
                # highway-weight loads interleaved after tg0/tg1 so layer-0
                # can start as soon as the first column group lands
                def emit_weights(l, wi):
                    wsrc, wdst = ((tW, tWh), (cW, cWh))[wi]
                    wv = wsrc[l].rearrange("(k p) m -> k p m", p=128)
                    for k in range(KU):
                        ws = stgw.tile([128, U], F32, tag="ws",
                                       name=f"ws_{l}_{wi}_{k}")
                        nc.sync.dma_start(ws[:, :U // 2], wv[k][:, :U // 2])
                        nc.sync.dma_start(ws[:, U // 2:], wv[k][:, U // 2:])
                        if k % 2 == 0:
                            nc.vector.tensor_copy(wdst[:, l, k, :], ws)
                        else:
                            nc.scalar.copy(wdst[:, l, k, :], ws)

                # warm the PE HAM clock-gate during the initial DMA wait:
                # ~40 tiny matmuls lift the PE to 2.4 GHz before the fp32
                # transposes (which never count as HAM-busy) begin
                warm = [ptA.tile([128, 512], F32, tag=f"ptk{k}",
                                 name=f"warm_{k}") for k in range(KU)]
                for i in range(40):
                    nc.tensor.matmul(warm[i % KU][:, 0:128], ident, ident,
                                     start=True, stop=True)
                for tg in range(NS):
                    ptk = [ptA.tile([128, 512], F32, tag=f"ptk{k}",
                                    name=f"ptk_{tg}_{k}")
                           for k in range(KU)]
                    for tt in range(4):
                        t = tg * 4 + tt
                        xs = stg.tile([128, U], F32, tag="xs",
                                      name=f"xs_{t}")
                        nc.sync.dma_start(xs[:, :U // 2], xv[t][:, :U // 2])
                        nc.sync.dma_start(xs[:, U // 2:], xv[t][:, U // 2:])
                        for k in range(KU):
                            nc.tensor.transpose(
                                ptk[k][:, tt * 128:(tt + 1) * 128],
                                xs[:, k * 128:(k + 1) * 128], identf)
                    for k in range(KU):
                        if k % 2 == 0:
                            nc.vector.tensor_copy(
                                xTh[:, k, tg * 512:(tg + 1) * 512], ptk[k])
                        else:
                            nc.scalar.copy(
                                xTh[:, k, tg * 512:(tg + 1) * 512], ptk[k])
                    if tg < H:
                        emit_weights(0, tg)
                    elif tg == H:
                        nc.sync.dma_start(
                            tbsb, tb.rearrange("l (m p) -> p l m", p=128))
                        nc.sync.dma_start(
                            cbsb, cb.rearrange("l (m p) -> p l m", p=128))
                        nc.sync.dma_start(
                            awsb, aW.rearrange("(w m p) -> p (w m)",
                                               p=128, w=3))
                        nc.vector.tensor_copy(w1h, awsb[:, 0:KU])
                        nc.vector.tensor_copy(w2h, awsb[:, KU:2 * KU])
                        nc.sync.dma_start(ab_sb, ab[None, :])
                        nc.scalar.mul(nab_sb, ab_sb, -1.0)
                        fb = stg.tile([1, U], F32, tag="fb")
                        nc.sync.dma_start(fb, ffb[None, :])
                        nc.vector.tensor_copy(ffb_h, fb)
                        fb2 = stg.tile([1, U], F32, tag="fb")
                        nc.sync.dma_start(fb2, frb[None, :])
                        nc.vector.tensor_copy(frb_h, fb2)
                    else:
                        emit_weights(1, 0)
                        emit_weights(1, 1)

            # ============= Phase B: highway stack (2 layers) ===========
            with tc.tile_pool(name="hwp", bufs=2, space="PSUM") as hwp, \
                 tc.tile_pool(name="hws", bufs=3) as hws:
                for l in range(H):
                    xin = xTh if l == 0 else x1T
                    xout = x1T if l == 0 else x2T
                    for t in range(NS):
                        nsl = slice(t * 512, (t + 1) * 512)
                        for m in range(KU):
                            pt = hwp.tile([128, 512], F32, tag="pt")
                            pc = hwp.tile([128, 512], F32, tag="pc")
                            for k in range(KU):
                                nc.tensor.matmul(
                                    pt, tWh[:, l, k, m * 128:(m + 1) * 128],
                                    xin[:, k, nsl],
                                    start=(k == 0), stop=(k == KU - 1))
                            for k in range(KU):
                                nc.tensor.matmul(
                                    pc, cWh[:, l, k, m * 128:(m + 1) * 128],
                                    xin[:, k, nsl],
                                    start=(k == 0), stop=(k == KU - 1))
                            th = hws.tile([128, 512], BF16, tag="th")
                            ch = hws.tile([128, 512], BF16, tag="ch")
                            nc.scalar.activation(
                                th, pt, AF.Relu, bias=tbsb[:, l, m:m + 1])
                            nc.scalar.activation(
                                ch, pc, AF.Sigmoid, bias=cbsb[:, l, m:m + 1])
                            dh = hws.tile([128, 512], BF16, tag="dh")
                            nc.vector.tensor_tensor(
                                dh, th, xin[:, m, nsl], op=OP.subtract)
                            mh = hws.tile([128, 512], BF16, tag="mh")
                            nc.vector.tensor_tensor(
                                mh, ch, dh, op=OP.mult)
                            nc.gpsimd.tensor_tensor(
                                xout[:, m, nsl], xin[:, m, nsl], mh,
                                op=OP.add)

            # ============= Phase C: attention prep =========================
            with tc.tile_pool(name="pcp", bufs=2, space="PSUM") as pcp, \
                 tc.tile_pool(name="pcp1", bufs=1, space="PSUM") as pcp1:
                # w3 * x^T  (w3 is per-partition here)
                for k in range(KU):
                    nc.vector.tensor_scalar_mul(
                        w3x[:, k, :], x2T[:, k, :], awsb[:, 8 + k:9 + k])
                # x back to row-major via PE transpose (bf16, psum staging)
                for jt in range(NT):
                    ptr = pcp.tile([128, 512], BF16, tag="ptr")
                    for k in range(KU):
                        nc.tensor.transpose(
                            ptr[:, k * 128:(k + 1) * 128],
                            x2T[:, k, jt * 128:(jt + 1) * 128], ident)
                    nc.vector.tensor_copy(xO[:, jt, :], ptr)
                # clamp threshold exp(-(s1+ab)); the factor exp(s1+ab)
                # cancels in the softmax so it never enters the matmuls
                for t in range(NS):
                    ps1 = pcp1.tile([1, 512], F32, tag="ps1")
                    for k in range(KU):
                        nc.tensor.matmul(ps1, w1h[:, k:k + 1],
                                         x2T[:, k, t * 512:(t + 1) * 512],
                                         start=(k == 0), stop=(k == KU - 1))
                    nc.scalar.activation(
                        thr[:, t * 512:(t + 1) * 512], ps1, AF.Exp,
                        bias=nab_sb, scale=-1.0)
                # s2 = x @ w2: all 16 j-tiles into one psum bank, one copy
                s2p = pcp1.tile([128, NT], F32, tag="s2p")
                for jt in range(NT):
                    for k in range(KU):
                        nc.tensor.matmul(s2p[:, jt:jt + 1],
                                         x2T[:, k, jt * 128:(jt + 1) * 128],
                                         w2h[:, k:k + 1],
                                         start=(k == 0), stop=(k == KU - 1))
                nc.vector.tensor_copy(s2f, s2p)

            # ============= Phase D: pairwise softmax attention =============
            fWv = ffW.rearrange("(k p) m -> k p m", p=128)
            rWv = frW.rearrange("(k p) m -> k p m", p=128)
            fuse_chunks = [(fWv, ffWh, k) for k in range(2 * KU)] + \
                          [(rWv, frWh, k) for k in range(2 * KU)]
            with tc.tile_pool(name="pdn", bufs=4, space="PSUM") as pdn, \
                 tc.tile_pool(name="pds", bufs=2, space="PSUM") as pds, \
                 tc.tile_pool(name="pdr", bufs=1, space="PSUM") as pdr, \
                 tc.tile_pool(name="pbc", bufs=1, space="PSUM") as pbc, \
                 tc.tile_pool(name="stgf", bufs=4) as stgf, \
                 tc.tile_pool(name="dsb", bufs=4) as dsb:
                for b in range(BPC):
                    for h in range(IH):
                        # drip-feed fuse-gate weight loads through the
                        # attention phase (gpsimd is idle here)
                        unit = b * IH + h
                        for ci in range(unit * 4, unit * 4 + 4):
                            wv_, wdst_, k_ = fuse_chunks[ci]
                            wsf = stgf.tile([128, U], F32, tag="wsf",
                                            name=f"wsf_{ci}")
                            nc.sync.dma_start(wsf[:, :U // 2],
                                              wv_[k_][:, :U // 2])
                            nc.sync.dma_start(wsf[:, U // 2:],
                                              wv_[k_][:, U // 2:])
                            if ci % 2 == 0:
                                nc.vector.tensor_copy(wdst_[:, k_, :], wsf)
                            else:
                                nc.scalar.copy(wdst_[:, k_, :], wsf)
                        isl = slice(b * L + h * 512, b * L + (h + 1) * 512)
                        pn = [pdn.tile([128, 512], F32, tag="pn",
                                       name=f"pn_{b}_{h}_{du}")
                              for du in range(KU)]
                        pr = pdr.tile([1, 512], F32, tag="pr")
                        thbc = dsb.tile([128, 512], BF16, tag="thbc")
                        pb1 = pbc.tile([128, 512], F32, tag="pb",
                                       name=f"pb1_{b}_{h}")
                        nc.tensor.matmul(pb1, ones_row, thr[:, isl],
                                         start=True, stop=True)
                        nc.scalar.copy(thbc, pb1)
                        for jt in range(JT):
                            jg = b * JT + jt
                            jsl = slice(b * L + jt * 128, b * L + (jt + 1) * 128)
                            ps = pds.tile([128, 512], F32, tag="ps")
                            for k in range(KU):
                                nc.tensor.matmul(ps, w3x[:, k, jsl],
                                                 x2T[:, k, isl],
                                                 start=(k == 0),
                                                 stop=(k == KU - 1))
                            eh = dsb.tile([128, 512], BF16, tag="eh")
                            nc.scalar.activation(eh, ps, AF.Exp,
                                                 bias=s2f[:, jg:jg + 1])
                            nc.vector.tensor_tensor(eh, eh, thbc, op=OP.max)
                            for du in range(KU):
                                nc.tensor.matmul(
                                    pn[du],
                                    xO[:, jg, du * 128:(du + 1) * 128], eh,
                                    start=(jt == 0), stop=(jt == JT - 1))
                            nc.tensor.matmul(pr, ones_col, eh,
                                             start=(jt == 0),
                                             stop=(jt == JT - 1))
                        rec = dsb.tile([1, 512], F32, tag="rec")
                        nc.vector.reciprocal_approx_fast(rec, pr)
                        rech = dsb.tile([1, 512], BF16, tag="rech")
                        nc.vector.tensor_copy(rech, rec)
                        rbc = dsb.tile([128, 512], BF16, tag="rbc")
                        pb2 = pbc.tile([128, 512], F32, tag="pb",
                                       name=f"pb2_{b}_{h}")
                        nc.tensor.matmul(pb2, ones_row, rech,
                                         start=True, stop=True)
                        nc.scalar.copy(rbc, pb2)
                        # drain pn psum banks quickly via scalar, then
                        # normalize in fast bf16 on vector
                        pnh = [dsb.tile([128, 512], BF16, tag="pnh",
                                        name=f"pnh_{b}_{h}_{du}")
                               for du in range(KU)]
                        for du in range(KU):
                            if du % 2 == 0:
                                nc.scalar.copy(pnh[du], pn[du])
                            else:
                                nc.vector.tensor_copy(pnh[du], pn[du])
                        for du in range(KU):
                            nc.vector.tensor_tensor(
                                attT[:, du, isl], pnh[du], rbc, op=OP.mult)

            # ============= Phase E: fuse gates + output ====================
            with tc.tile_pool(name="pep", bufs=2, space="PSUM") as pep, \
                 tc.tile_pool(name="esb", bufs=3) as esb:
                for mt in range(NT):
                    msl = slice(mt * 128, (mt + 1) * 128)
                    x0t = esb.tile([128, U], F32, tag="x0t")
                    nc.sync.dma_start(x0t[:, :U // 2], xv[mt][:, :U // 2])
                    nc.sync.dma_start(x0t[:, U // 2:], xv[mt][:, U // 2:])
                    pz = pep.tile([128, 512], F32, tag="pz")
                    pr2 = pep.tile([128, 512], F32, tag="pr2")
                    for k in range(2 * KU):
                        lhsT = (xTh[:, k, msl] if k < KU
                                else attT[:, k - KU, msl])
                        nc.tensor.matmul(pz, lhsT, ffWh[:, k, :],
                                         start=(k == 0), stop=False)
                        nc.tensor.matmul(pr2, lhsT, frWh[:, k, :],
                                         start=(k == 0), stop=False)
                    nc.tensor.matmul(pz, ones_row, ffb_h, start=False,
                                     stop=True)
                    nc.tensor.matmul(pr2, ones_row, frb_h, start=False,
                                     stop=True)
                    zh = esb.tile([128, U], BF16, tag="zh")
                    rh = esb.tile([128, U], BF16, tag="rh")
                    nc.scalar.activation(zh, pz, AF.Sigmoid)
                    nc.scalar.activation(rh, pr2, AF.Sigmoid)
                    q = esb.tile([128, U], F32, tag="q")
                    nc.gpsimd.tensor_tensor(q, zh, zh, op=OP.mult)
                    p2 = esb.tile([128, U], F32, tag="p2")
                    nc.vector.tensor_tensor(p2, rh, x0t, op=OP.mult)
                    ot = esb.tile([128, U], F32, tag="ot")
                    nc.vector.tensor_tensor(ot, q, p2, op=OP.add)
                    for qq in range(4):
                        csl = slice(qq * (U // 4), (qq + 1) * (U // 4))
                        nc.sync.dma_start(outv[mt][:, csl], ot[:, csl])

    nc.compile()
    return nc


_NC_CACHE = None


def _get_nc():
    global _NC_CACHE
    if _NC_CACHE is None:
        _NC_CACHE = build_nc()
    return _NC_CACHE


def kernel(**inputs) -> np.ndarray:
    from concourse.bass_utils import run_bass_kernel_spmd

    nc = _get_nc()
    full = {k: np.ascontiguousarray(np.asarray(v, dtype=np.float32))
            for k, v in inputs.items()}
    in_maps = []
    for c in range(NCORES):
        m = dict(full)
        m["inputs"] = np.ascontiguousarray(
            full["inputs"][c * BPC:(c + 1) * BPC])
        in_maps.append(m)
    res = run_bass_kernel_spmd(nc, in_maps, core_ids=list(range(NCORES)))
    return np.concatenate([res.results[c]["out"] for c in range(NCORES)],
                          axis=0)


# revision 33
# speedup vs baseline: 1.0491x; 1.0491x over previous
"""Trainium2 Bass kernel for nn_Encoding_layer (highway stack + pairwise MLP
attention + fuse gates).

Sharding: data-parallel over batch B=16 across 8 NeuronCores (2 batches per
core); all dense weights replicated. No collectives.

Per-core layouts (n = 2 batches x L=1024 = 2048 token-columns):
  xTh/x1T/x2T/w3x/attT : [128, 4, 2048] bf16, "transposed" activations
                         [u mod 128, u div 128, n]
  xO                   : [128, 16, 512] bf16, row-major highway output
                         [row mod 128, row div 128, u]
  Attention: S^T[j,i] = s3[j,i] (PE, w3*x^T as lhsT) + s2[j] (ACT exp bias).
  The per-column term s1[i]+ab never enters the matmuls: exp(S+s1+ab) =
  exp(s1+ab)*exp(S), and a per-column factor cancels in the softmax, so
  relu becomes a clamp against th[i] = exp(-(s1[i]+ab)):
      M^T = max(exp(s3+s2), th)  ==  exp(relu(S_full)) / exp(s1+ab)
  Numerator att^T (lhsT = row-major x) and denominator r (lhsT = ones
  column) come from matmuls against M^T; normalization multiplies by the
  broadcast fast-approx reciprocal of r.  Broadcasts of [1,512] rows are
  PE outer-products (ones_row as lhsT) + scalar-engine copies - gpsimd
  partition_broadcast triggers multi-us ucode LIBRARY_RELOAD stalls.
"""

import numpy as np

B, L, U, H = 16, 1024, 512, 2
NCORES = 8
BPC = B // NCORES          # batches per core
N = BPC * L                # token columns per core
KU = U // 128              # 4  u-tiles
NT = N // 128              # 16 row-tiles per core
NS = N // 512              # 4  512-wide column slices per core
JT = L // 128              # 8  j-tiles per batch
IH = L // 512              # 2  i-halves per batch


def build_nc():
    import concourse.bacc as bacc
    import concourse.tile as tile
    from concourse import mybir
    from concourse.masks import make_identity

    F32 = mybir.dt.float32
    BF16 = mybir.dt.bfloat16
    AF = mybir.ActivationFunctionType
    OP = mybir.AluOpType

    nc = bacc.Bacc("TRN2", target_bir_lowering=False, debug=False,
                   num_devices=NCORES)

    x_in = nc.dram_tensor("inputs", [BPC, L, U], F32, kind="ExternalInput").ap()
    tW = nc.dram_tensor("tW", [H, U, U], F32, kind="ExternalInput").ap()
    tb = nc.dram_tensor("tb", [H, U], F32, kind="ExternalInput").ap()
    cW = nc.dram_tensor("cW", [H, U, U], F32, kind="ExternalInput").ap()
    cb = nc.dram_tensor("cb", [H, U], F32, kind="ExternalInput").ap()
    aW = nc.dram_tensor("aW", [3 * U], F32, kind="ExternalInput").ap()
    ab = nc.dram_tensor("ab", [1], F32, kind="ExternalInput").ap()
    frW = nc.dram_tensor("frW", [2 * U, U], F32, kind="ExternalInput").ap()
    frb = nc.dram_tensor("frb", [U], F32, kind="ExternalInput").ap()
    ffW = nc.dram_tensor("ffW", [2 * U, U], F32, kind="ExternalInput").ap()
    ffb = nc.dram_tensor("ffb", [U], F32, kind="ExternalInput").ap()
    out = nc.dram_tensor("out", [BPC, L, U], F32, kind="ExternalOutput").ap()

    xv = x_in.flatten_outer_dims().rearrange("(t p) u -> t p u", p=128)
    outv = out.flatten_outer_dims().rearrange("(t p) u -> t p u", p=128)

    with tile.TileContext(nc) as tc:
        with tc.tile_pool(name="pers", bufs=1) as pers:
            # ---- persistent SBUF tensors ----
            xTh = pers.tile([128, KU, N], BF16, tag="xTh")    # inputs^T
            x1T = pers.tile([128, KU, N], BF16, tag="x1T")
            x2T = pers.tile([128, KU, N], BF16, tag="x2T")
            w3x = pers.tile([128, KU, N], BF16, tag="w3x")
            attT = pers.tile([128, KU, N], BF16, tag="attT")
            xO = pers.tile([128, NT, U], BF16, tag="xO")
            tWh = pers.tile([128, H, KU, U], BF16, tag="tWh")
            cWh = pers.tile([128, H, KU, U], BF16, tag="cWh")
            ffWh = pers.tile([128, 2 * KU, U], BF16, tag="ffWh")
            frWh = pers.tile([128, 2 * KU, U], BF16, tag="frWh")
            tbsb = pers.tile([128, H, KU], F32, tag="tbsb")
            cbsb = pers.tile([128, H, KU], F32, tag="cbsb")
            awsb = pers.tile([128, 12], F32, tag="awsb")      # w1|w2|w3 cols
            w1h = pers.tile([128, KU], BF16, tag="w1h")
            w2h = pers.tile([128, KU], BF16, tag="w2h")
            ab_sb = pers.tile([1, 1], F32, tag="ab_sb")
            nab_sb = pers.tile([1, 1], F32, tag="nab_sb")
            ffb_h = pers.tile([1, U], BF16, tag="ffb_h")
            frb_h = pers.tile([1, U], BF16, tag="frb_h")
            thr = pers.tile([1, N], BF16, tag="thr")   # exp(-(s1+ab))
            s2f = pers.tile([128, NT], F32, tag="s2f")
            ones_row = pers.tile([1, 128], BF16, tag="ones_row")
            ones_col = pers.tile([128, 1], BF16, tag="ones_col")
            ident = pers.tile([128, 128], BF16, tag="ident")
            identf = pers.tile([128, 128], F32, tag="identf")

            nc.vector.memset(ones_row, 1.0)
            nc.vector.memset(ones_col, 1.0)
            make_identity(nc, ident)
            make_identity(nc, identf)

            # ================= Phase A: loads, casts, input transpose ======
            with tc.tile_pool(name="stg", bufs=8) as stg, \
                 tc.tile_pool(name="stgw", bufs=8) as stgw, \
                 tc.tile_pool(name="ptA", bufs=1, space="PSUM") as ptA:
                # inputs^T via PE transpose (PE is idle here), with
                # highway-weight loads interleaved after tg0/tg1 so layer-0
                # can start as soon as the first column group lands
                def emit_weights(l, wi):
                    wsrc, wdst = ((tW, tWh), (cW, cWh))[wi]
                    wv = wsrc[l].rearrange("(k p) m -> k p m", p=128)
                    for k in range(KU):
                        ws = stgw.tile([128, U], F32, tag="ws",
                                       name=f"ws_{l}_{wi}_{k}")
                        nc.sync.dma_start(ws, wv[k])
                        if k % 2 == 0:
                            nc.vector.tensor_copy(wdst[:, l, k, :], ws)
                        else:
                            nc.scalar.copy(wdst[:, l, k, :], ws)

                # warm the PE HAM clock-gate during the initial DMA wait:
                # ~40 tiny matmuls lift the PE to 2.4 GHz before the fp32
                # transposes (which never count as HAM-busy) begin
                warm = [ptA.tile([128, 512], F32, tag=f"ptk{k}",
                                 name=f"warm_{k}") for k in range(KU)]
                for i in range(40):
                    nc.tensor.matmul(warm[i % KU][:, 0:128], ident, ident,
                                     start=True, stop=True)
                for tg in range(NS):
                    ptk = [ptA.tile([128, 512], F32, tag=f"ptk{k}",
                                    name=f"ptk_{tg}_{k}")
                           for k in range(KU)]
                    for tt in range(4):
                        t = tg * 4 + tt
                        xs = stg.tile([128, U], F32, tag="xs",
                                      name=f"xs_{t}")
                        nc.sync.dma_start(xs, xv[t])
                        for k in range(KU):
                            nc.tensor.transpose(
                                ptk[k][:, tt * 128:(tt + 1) * 128],
                                xs[:, k * 128:(k + 1) * 128], identf)
                    for k in range(KU):
                        if k % 2 == 0:
                            nc.vector.tensor_copy(
                                xTh[:, k, tg * 512:(tg + 1) * 512], ptk[k])
                        else:
                            nc.scalar.copy(
                                xTh[:, k, tg * 512:(tg + 1) * 512], ptk[k])
                    if tg < H:
                        emit_weights(0, tg)
                    elif tg == H:
                        nc.sync.dma_start(
                            tbsb, tb.rearrange("l (m p) -> p l m", p=128))
                        nc.sync.dma_start(
                            cbsb, cb.rearrange("l (m p) -> p l m", p=128))
                        nc.sync.dma_start(
                            awsb, aW.rearrange("(w m p) -> p (w m)",
                                               p=128, w=3))
                        nc.vector.tensor_copy(w1h, awsb[:, 0:KU])
                        nc.vector.tensor_copy(w2h, awsb[:, KU:2 * KU])
                        nc.sync.dma_start(ab_sb, ab[None, :])
                        nc.scalar.mul(nab_sb, ab_sb, -1.0)
                        fb = stg.tile([1, U], F32, tag="fb")
                        nc.sync.dma_start(fb, ffb[None, :])
                        nc.vector.tensor_copy(ffb_h, fb)
                        fb2 = stg.tile([1, U], F32, tag="fb")
                        nc.sync.dma_start(fb2, frb[None, :])
                        nc.vector.tensor_copy(frb_h, fb2)
                    else:
                        emit_weights(1, 0)
                        emit_weights(1, 1)

            # ============= Phase B: highway stack (2 layers) ===========
            with tc.tile_pool(name="hwp", bufs=2, space="PSUM") as hwp, \
                 tc.tile_pool(name="hws", bufs=3) as hws:
                for l in range(H):
                    xin = xTh if l == 0 else x1T
                    xout = x1T if l == 0 else x2T
                    for t in range(NS):
                        nsl = slice(t * 512, (t + 1) * 512)
                        for m in range(KU):
                            pt = hwp.tile([128, 512], F32, tag="pt")
                            pc = hwp.tile([128, 512], F32, tag="pc")
                            for k in range(KU):
                                nc.tensor.matmul(
                                    pt, tWh[:, l, k, m * 128:(m + 1) * 128],
                                    xin[:, k, nsl],
                                    start=(k == 0), stop=(k == KU - 1))
                            for k in range(KU):
                                nc.tensor.matmul(
                                    pc, cWh[:, l, k, m * 128:(m + 1) * 128],
                                    xin[:, k, nsl],
                                    start=(k == 0), stop=(k == KU - 1))
                            th = hws.tile([128, 512], BF16, tag="th")
                            ch = hws.tile([128, 512], BF16, tag="ch")
                            nc.scalar.activation(
                                th, pt, AF.Relu, bias=tbsb[:, l, m:m + 1])
                            nc.scalar.activation(
                                ch, pc, AF.Sigmoid, bias=cbsb[:, l, m:m + 1])
                            dh = hws.tile([128, 512], BF16, tag="dh")
                            nc.vector.tensor_tensor(
                                dh, th, xin[:, m, nsl], op=OP.subtract)
                            mh = hws.tile([128, 512], BF16, tag="mh")
                            nc.vector.tensor_tensor(
                                mh, ch, dh, op=OP.mult)
                            nc.gpsimd.tensor_tensor(
                                xout[:, m, nsl], xin[:, m, nsl], mh,
                                op=OP.add)

            # ============= Phase C: attention prep =========================
            with tc.tile_pool(name="pcp", bufs=2, space="PSUM") as pcp, \
                 tc.tile_pool(name="pcp1", bufs=1, space="PSUM") as pcp1:
                # w3 * x^T  (w3 is per-partition here)
                for k in range(KU):
                    nc.vector.tensor_scalar_mul(
                        w3x[:, k, :], x2T[:, k, :], awsb[:, 8 + k:9 + k])
                # x back to row-major via PE transpose (bf16, psum staging)
                for jt in range(NT):
                    ptr = pcp.tile([128, 512], BF16, tag="ptr")
                    for k in range(KU):
                        nc.tensor.transpose(
                            ptr[:, k * 128:(k + 1) * 128],
                            x2T[:, k, jt * 128:(jt + 1) * 128], ident)
                    nc.vector.tensor_copy(xO[:, jt, :], ptr)
                # clamp threshold exp(-(s1+ab)); the factor exp(s1+ab)
                # cancels in the softmax so it never enters the matmuls
                for t in range(NS):
                    ps1 = pcp1.tile([1, 512], F32, tag="ps1")
                    for k in range(KU):
                        nc.tensor.matmul(ps1, w1h[:, k:k + 1],
                                         x2T[:, k, t * 512:(t + 1) * 512],
                                         start=(k == 0), stop=(k == KU - 1))
                    nc.scalar.activation(
                        thr[:, t * 512:(t + 1) * 512], ps1, AF.Exp,
                        bias=nab_sb, scale=-1.0)
                # s2 = x @ w2: all 16 j-tiles into one psum bank, one copy
                s2p = pcp1.tile([128, NT], F32, tag="s2p")
                for jt in range(NT):
                    for k in range(KU):
                        nc.tensor.matmul(s2p[:, jt:jt + 1],
                                         x2T[:, k, jt * 128:(jt + 1) * 128],
                                         w2h[:, k:k + 1],
                                         start=(k == 0), stop=(k == KU - 1))
                nc.vector.tensor_copy(s2f, s2p)

            # ============= Phase D: pairwise softmax attention =============
            fWv = ffW.rearrange("(k p) m -> k p m", p=128)
            rWv = frW.rearrange("(k p) m -> k p m", p=128)
            fuse_chunks = [(fWv, ffWh, k) for k in range(2 * KU)] + \
                          [(rWv, frWh, k) for k in range(2 * KU)]
            with tc.tile_pool(name="pdn", bufs=4, space="PSUM") as pdn, \
                 tc.tile_pool(name="pds", bufs=2, space="PSUM") as pds, \
                 tc.tile_pool(name="pdr", bufs=1, space="PSUM") as pdr, \
                 tc.tile_pool(name="pbc", bufs=1, space="PSUM") as pbc, \
                 tc.tile_pool(name="stgf", bufs=4) as stgf, \
                 tc.tile_pool(name="dsb", bufs=4) as dsb:
                for b in range(BPC):
                    for h in range(IH):
                        # drip-feed fuse-gate weight loads through the
                        # attention phase (gpsimd is idle here)
                        unit = b * IH + h
                        for ci in range(unit * 4, unit * 4 + 4):
                            wv_, wdst_, k_ = fuse_chunks[ci]
                            wsf = stgf.tile([128, U], F32, tag="wsf",
                                            name=f"wsf_{ci}")
                            nc.sync.dma_start(wsf, wv_[k_])
                            if ci % 2 == 0:
                                nc.vector.tensor_copy(wdst_[:, k_, :], wsf)
                            else:
                                nc.scalar.copy(wdst_[:, k_, :], wsf)
                        isl = slice(b * L + h * 512, b * L + (h + 1) * 512)
                        pn = [pdn.tile([128, 512], F32, tag="pn",
                                       name=f"pn_{b}_{h}_{du}")
                              for du in range(KU)]
                        pr = pdr.tile([1, 512], F32, tag="pr")
                        thbc = dsb.tile([128, 512], BF16, tag="thbc")
                        pb1 = pbc.tile([128, 512], F32, tag="pb",
                                       name=f"pb1_{b}_{h}")
                        nc.tensor.matmul(pb1, ones_row, thr[:, isl],
                                         start=True, stop=True)
                        nc.scalar.copy(thbc, pb1)
                        for jt in range(JT):
                            jg = b * JT + jt
                            jsl = slice(b * L + jt * 128, b * L + (jt + 1) * 128)
                            ps = pds.tile([128, 512], F32, tag="ps")
                            for k in range(KU):
                                nc.tensor.matmul(ps, w3x[:, k, jsl],
                                                 x2T[:, k, isl],
                                                 start=(k == 0),
                                                 stop=(k == KU - 1))
                            eh = dsb.tile([128, 512], BF16, tag="eh")
                            nc.scalar.activation(eh, ps, AF.Exp,
                                                 bias=s2f[:, jg:jg + 1])
                            nc.vector.tensor_tensor(eh, eh, thbc, op=OP.max)
                            for du in range(KU):
                                nc.tensor.matmul(
                                    pn[du],
                                    xO[:, jg, du * 128:(du + 1) * 128], eh,
                                    start=(jt == 0), stop=(jt == JT - 1))
                            nc.tensor.matmul(pr, ones_col, eh,
                                             start=(jt == 0),
                                             stop=(jt == JT - 1))
                        rec = dsb.tile([1, 512], F32, tag="rec")
                        nc.vector.reciprocal_approx_fast(rec, pr)
                        rech = dsb.tile([1, 512], BF16, tag="rech")
                        nc.vector.tensor_copy(rech, rec)
                        rbc = dsb.tile([128, 512], BF16, tag="rbc")
                        pb2 = pbc.tile([128, 512], F32, tag="pb",
                                       name=f"pb2_{b}_{h}")
                        nc.tensor.matmul(pb2, ones_row, rech,
                                         start=True, stop=True)
                        nc.scalar.copy(rbc, pb2)
                        # drain pn psum banks quickly via scalar, then
                        # normalize in fast bf16 on vector
                        pnh = [dsb.tile([128, 512], BF16, tag="pnh",
                                        name=f"pnh_{b}_{h}_{du}")
                               for du in range(KU)]
                        for du in range(KU):
                            if du % 2 == 0:
                                nc.scalar.copy(pnh[du], pn[du])
                            else:
                                nc.vector.tensor_copy(pnh[du], pn[du])
                        for du in range(KU):
                            nc.vector.tensor_tensor(
                                attT[:, du, isl], pnh[du], rbc, op=OP.mult)

            # ============= Phase E: fuse gates + output ====================
            with tc.tile_pool(name="pep", bufs=2, space="PSUM") as pep, \
                 tc.tile_pool(name="esb", bufs=3) as esb:
                for mt in range(NT):
                    msl = slice(mt * 128, (mt + 1) * 128)
                    x0t = esb.tile([128, U], F32, tag="x0t")
                    nc.sync.dma_start(x0t, xv[mt])
                    pz = pep.tile([128, 512], F32, tag="pz")
                    pr2 = pep.tile([128, 512], F32, tag="pr2")
                    for k in range(2 * KU):
                        lhsT = (xTh[:, k, msl] if k < KU
                                else attT[:, k - KU, msl])
                        nc.tensor.matmul(pz, lhsT, ffWh[:, k, :],
                                         start=(k == 0), stop=False)
                        nc.tensor.matmul(pr2, lhsT, frWh[:, k, :],
                                         start=(k == 0), stop=False)
                    nc.tensor.matmul(pz, ones_row, ffb_h, start=False,
                                     stop=True)
                    nc.tensor.matmul(pr2, ones_row, frb_h, start=False,
                                     stop=True)
                    zh = esb.tile([128, U], BF16, tag="zh")
                    rh = esb.tile([128, U], BF16, tag="rh")
                    nc.scalar.activation(zh, pz, AF.Sigmoid)
                    nc.scalar.activation(rh, pr2, AF.Sigmoid)
                    q = esb.tile([128, U], F32, tag="q")
                    nc.gpsimd.tensor_tensor(q, zh, zh, op=OP.mult)
                    p2 = esb.tile([128, U], F32, tag="p2")
                    nc.vector.tensor_tensor(p2, rh, x0t, op=OP.mult)
                    ot = esb.tile([128, U], F32, tag="ot")
                    nc.vector.tensor_tensor(ot, q, p2, op=OP.add)
                    nc.sync.dma_start(outv[mt], ot)

    nc.compile()
    return nc


_NC_CACHE = None


def _get_nc():
    global _NC_CACHE
    if _NC_CACHE is None:
        _NC_CACHE = build_nc()
    return _NC_CACHE


def kernel(**inputs) -> np.ndarray:
    from concourse.bass_utils import run_bass_kernel_spmd

    nc = _get_nc()
    full = {k: np.ascontiguousarray(np.asarray(v, dtype=np.float32))
            for k, v in inputs.items()}
    in_maps = []
    for c in range(NCORES):
        m = dict(full)
        m["inputs"] = np.ascontiguousarray(
            full["inputs"][c * BPC:(c + 1) * BPC])
        in_maps.append(m)
    res = run_bass_kernel_spmd(nc, in_maps, core_ids=list(range(NCORES)))
    return np.concatenate([res.results[c]["out"] for c in range(NCORES)],
                          axis=0)


# revision 34
# speedup vs baseline: 1.0507x; 1.0015x over previous
"""Trainium2 Bass kernel for nn_Encoding_layer (highway stack + pairwise MLP
attention + fuse gates).

Sharding: data-parallel over batch B=16 across 8 NeuronCores (2 batches per
core); all dense weights replicated. No collectives.

Per-core layouts (n = 2 batches x L=1024 = 2048 token-columns):
  xTh/x1T/x2T/w3x/attT : [128, 4, 2048] bf16, "transposed" activations
                         [u mod 128, u div 128, n]
  xO                   : [128, 16, 512] bf16, row-major highway output
                         [row mod 128, row div 128, u]
  Attention: S^T[j,i] = s3[j,i] (PE, w3*x^T as lhsT) + s2[j] (ACT exp bias).
  The per-column term s1[i]+ab never enters the matmuls: exp(S+s1+ab) =
  exp(s1+ab)*exp(S), and a per-column factor cancels in the softmax, so
  relu becomes a clamp against th[i] = exp(-(s1[i]+ab)):
      M^T = max(exp(s3+s2), th)  ==  exp(relu(S_full)) / exp(s1+ab)
  Numerator att^T (lhsT = row-major x) and denominator r (lhsT = ones
  column) come from matmuls against M^T; normalization multiplies by the
  broadcast fast-approx reciprocal of r.  Broadcasts of [1,512] rows are
  PE outer-products (ones_row as lhsT) + scalar-engine copies - gpsimd
  partition_broadcast triggers multi-us ucode LIBRARY_RELOAD stalls.
"""

import numpy as np

B, L, U, H = 16, 1024, 512, 2
NCORES = 8
BPC = B // NCORES          # batches per core
N = BPC * L                # token columns per core
KU = U // 128              # 4  u-tiles
NT = N // 128              # 16 row-tiles per core
NS = N // 512              # 4  512-wide column slices per core
JT = L // 128              # 8  j-tiles per batch
IH = L // 512              # 2  i-halves per batch


def build_nc():
    import concourse.bacc as bacc
    import concourse.tile as tile
    from concourse import mybir
    from concourse.masks import make_identity

    F32 = mybir.dt.float32
    BF16 = mybir.dt.bfloat16
    AF = mybir.ActivationFunctionType
    OP = mybir.AluOpType

    nc = bacc.Bacc("TRN2", target_bir_lowering=False, debug=False,
                   num_devices=NCORES)

    x_in = nc.dram_tensor("inputs", [BPC, L, U], F32, kind="ExternalInput").ap()
    tW = nc.dram_tensor("tW", [H, U, U], F32, kind="ExternalInput").ap()
    tb = nc.dram_tensor("tb", [H, U], F32, kind="ExternalInput").ap()
    cW = nc.dram_tensor("cW", [H, U, U], F32, kind="ExternalInput").ap()
    cb = nc.dram_tensor("cb", [H, U], F32, kind="ExternalInput").ap()
    aW = nc.dram_tensor("aW", [3 * U], F32, kind="ExternalInput").ap()
    ab = nc.dram_tensor("ab", [1], F32, kind="ExternalInput").ap()
    frW = nc.dram_tensor("frW", [2 * U, U], F32, kind="ExternalInput").ap()
    frb = nc.dram_tensor("frb", [U], F32, kind="ExternalInput").ap()
    ffW = nc.dram_tensor("ffW", [2 * U, U], F32, kind="ExternalInput").ap()
    ffb = nc.dram_tensor("ffb", [U], F32, kind="ExternalInput").ap()
    out = nc.dram_tensor("out", [BPC, L, U], F32, kind="ExternalOutput").ap()

    xv = x_in.flatten_outer_dims().rearrange("(t p) u -> t p u", p=128)
    outv = out.flatten_outer_dims().rearrange("(t p) u -> t p u", p=128)

    with tile.TileContext(nc) as tc:
        with tc.tile_pool(name="pers", bufs=1) as pers:
            # ---- persistent SBUF tensors ----
            xTh = pers.tile([128, KU, N], BF16, tag="xTh")    # inputs^T
            x1T = pers.tile([128, KU, N], BF16, tag="x1T")
            x2T = pers.tile([128, KU, N], BF16, tag="x2T")
            w3x = pers.tile([128, KU, N], BF16, tag="w3x")
            attT = pers.tile([128, KU, N], BF16, tag="attT")
            xO = pers.tile([128, NT, U], BF16, tag="xO")
            tWh = pers.tile([128, H, KU, U], BF16, tag="tWh")
            cWh = pers.tile([128, H, KU, U], BF16, tag="cWh")
            ffWh = pers.tile([128, 2 * KU, U], BF16, tag="ffWh")
            frWh = pers.tile([128, 2 * KU, U], BF16, tag="frWh")
            tbsb = pers.tile([128, H, KU], F32, tag="tbsb")
            cbsb = pers.tile([128, H, KU], F32, tag="cbsb")
            awsb = pers.tile([128, 12], F32, tag="awsb")      # w1|w2|w3 cols
            w1h = pers.tile([128, KU], BF16, tag="w1h")
            w2h = pers.tile([128, KU], BF16, tag="w2h")
            ab_sb = pers.tile([1, 1], F32, tag="ab_sb")
            nab_sb = pers.tile([1, 1], F32, tag="nab_sb")
            ffb_h = pers.tile([1, U], BF16, tag="ffb_h")
            frb_h = pers.tile([1, U], BF16, tag="frb_h")
            thr = pers.tile([1, N], BF16, tag="thr")   # exp(-(s1+ab))
            s2f = pers.tile([128, NT], F32, tag="s2f")
            ones_row = pers.tile([1, 128], BF16, tag="ones_row")
            ones_col = pers.tile([128, 1], BF16, tag="ones_col")
            ident = pers.tile([128, 128], BF16, tag="ident")
            identf = pers.tile([128, 128], F32, tag="identf")

            nc.vector.memset(ones_row, 1.0)
            nc.vector.memset(ones_col, 1.0)
            make_identity(nc, ident)
            make_identity(nc, identf)

            # ================= Phase A: loads, casts, input transpose ======
            with tc.tile_pool(name="stg", bufs=8) as stg, \
                 tc.tile_pool(name="stgw", bufs=8) as stgw, \
                 tc.tile_pool(name="ptA", bufs=1, space="PSUM") as ptA:
                # inputs^T via PE transpose (PE is idle here), with
                # highway-weight loads interleaved after tg0/tg1 so layer-0
                # can start as soon as the first column group lands
                def emit_weights(l, wi):
                    wsrc, wdst = ((tW, tWh), (cW, cWh))[wi]
                    wv = wsrc[l].rearrange("(k p) m -> k p m", p=128)
                    for k in range(KU):
                        ws = stgw.tile([128, U], F32, tag="ws",
                                       name=f"ws_{l}_{wi}_{k}")
                        nc.sync.dma_start(ws, wv[k])
                        if k % 2 == 0:
                            nc.vector.tensor_copy(wdst[:, l, k, :], ws)
                        else:
                            nc.scalar.copy(wdst[:, l, k, :], ws)

                # warm the PE HAM clock-gate during the initial DMA wait:
                # ~40 tiny matmuls lift the PE to 2.4 GHz before the fp32
                # transposes (which never count as HAM-busy) begin
                warm = [ptA.tile([128, 512], F32, tag=f"ptk{k}",
                                 name=f"warm_{k}") for k in range(KU)]
                for i in range(56):
                    nc.tensor.matmul(warm[i % KU][:, 0:128], ident, ident,
                                     start=True, stop=True)
                for tg in range(NS):
                    ptk = [ptA.tile([128, 512], F32, tag=f"ptk{k}",
                                    name=f"ptk_{tg}_{k}")
                           for k in range(KU)]
                    for tt in range(4):
                        t = tg * 4 + tt
                        xs = stg.tile([128, U], F32, tag="xs",
                                      name=f"xs_{t}")
                        nc.sync.dma_start(xs, xv[t])
                        for k in range(KU):
                            nc.tensor.transpose(
                                ptk[k][:, tt * 128:(tt + 1) * 128],
                                xs[:, k * 128:(k + 1) * 128], identf)
                    for k in range(KU):
                        if k % 2 == 0:
                            nc.vector.tensor_copy(
                                xTh[:, k, tg * 512:(tg + 1) * 512], ptk[k])
                        else:
                            nc.scalar.copy(
                                xTh[:, k, tg * 512:(tg + 1) * 512], ptk[k])
                    if tg < H:
                        emit_weights(0, tg)
                    elif tg == H:
                        nc.sync.dma_start(
                            tbsb, tb.rearrange("l (m p) -> p l m", p=128))
                        nc.sync.dma_start(
                            cbsb, cb.rearrange("l (m p) -> p l m", p=128))
                        nc.sync.dma_start(
                            awsb, aW.rearrange("(w m p) -> p (w m)",
                                               p=128, w=3))
                        nc.vector.tensor_copy(w1h, awsb[:, 0:KU])
                        nc.vector.tensor_copy(w2h, awsb[:, KU:2 * KU])
                        nc.sync.dma_start(ab_sb, ab[None, :])
                        nc.scalar.mul(nab_sb, ab_sb, -1.0)
                        fb = stg.tile([1, U], F32, tag="fb")
                        nc.sync.dma_start(fb, ffb[None, :])
                        nc.vector.tensor_copy(ffb_h, fb)
                        fb2 = stg.tile([1, U], F32, tag="fb")
                        nc.sync.dma_start(fb2, frb[None, :])
                        nc.vector.tensor_copy(frb_h, fb2)
                    else:
                        emit_weights(1, 0)
                        emit_weights(1, 1)

            # ============= Phase B: highway stack (2 layers) ===========
            with tc.tile_pool(name="hwp", bufs=2, space="PSUM") as hwp, \
                 tc.tile_pool(name="hws", bufs=3) as hws:
                for l in range(H):
                    xin = xTh if l == 0 else x1T
                    xout = x1T if l == 0 else x2T
                    for t in range(NS):
                        nsl = slice(t * 512, (t + 1) * 512)
                        for m in range(KU):
                            pt = hwp.tile([128, 512], F32, tag="pt")
                            pc = hwp.tile([128, 512], F32, tag="pc")
                            for k in range(KU):
                                nc.tensor.matmul(
                                    pt, tWh[:, l, k, m * 128:(m + 1) * 128],
                                    xin[:, k, nsl],
                                    start=(k == 0), stop=(k == KU - 1))
                            for k in range(KU):
                                nc.tensor.matmul(
                                    pc, cWh[:, l, k, m * 128:(m + 1) * 128],
                                    xin[:, k, nsl],
                                    start=(k == 0), stop=(k == KU - 1))
                            th = hws.tile([128, 512], BF16, tag="th")
                            ch = hws.tile([128, 512], BF16, tag="ch")
                            nc.scalar.activation(
                                th, pt, AF.Relu, bias=tbsb[:, l, m:m + 1])
                            nc.scalar.activation(
                                ch, pc, AF.Sigmoid, bias=cbsb[:, l, m:m + 1])
                            dh = hws.tile([128, 512], BF16, tag="dh")
                            nc.vector.tensor_tensor(
                                dh, th, xin[:, m, nsl], op=OP.subtract)
                            mh = hws.tile([128, 512], BF16, tag="mh")
                            nc.vector.tensor_tensor(
                                mh, ch, dh, op=OP.mult)
                            nc.gpsimd.tensor_tensor(
                                xout[:, m, nsl], xin[:, m, nsl], mh,
                                op=OP.add)

            # ============= Phase C: attention prep =========================
            with tc.tile_pool(name="pcp", bufs=2, space="PSUM") as pcp, \
                 tc.tile_pool(name="pcp1", bufs=1, space="PSUM") as pcp1:
                # w3 * x^T  (w3 is per-partition here)
                for k in range(KU):
                    nc.vector.tensor_scalar_mul(
                        w3x[:, k, :], x2T[:, k, :], awsb[:, 8 + k:9 + k])
                # x back to row-major via PE transpose (bf16, psum staging)
                for jt in range(NT):
                    ptr = pcp.tile([128, 512], BF16, tag="ptr")
                    for k in range(KU):
                        nc.tensor.transpose(
                            ptr[:, k * 128:(k + 1) * 128],
                            x2T[:, k, jt * 128:(jt + 1) * 128], ident)
                    nc.vector.tensor_copy(xO[:, jt, :], ptr)
                # clamp threshold exp(-(s1+ab)); the factor exp(s1+ab)
                # cancels in the softmax so it never enters the matmuls
                for t in range(NS):
                    ps1 = pcp1.tile([1, 512], F32, tag="ps1")
                    for k in range(KU):
                        nc.tensor.matmul(ps1, w1h[:, k:k + 1],
                                         x2T[:, k, t * 512:(t + 1) * 512],
                                         start=(k == 0), stop=(k == KU - 1))
                    nc.scalar.activation(
                        thr[:, t * 512:(t + 1) * 512], ps1, AF.Exp,
                        bias=nab_sb, scale=-1.0)
                # s2 = x @ w2: all 16 j-tiles into one psum bank, one copy
                s2p = pcp1.tile([128, NT], F32, tag="s2p")
                for jt in range(NT):
                    for k in range(KU):
                        nc.tensor.matmul(s2p[:, jt:jt + 1],
                                         x2T[:, k, jt * 128:(jt + 1) * 128],
                                         w2h[:, k:k + 1],
                                         start=(k == 0), stop=(k == KU - 1))
                nc.vector.tensor_copy(s2f, s2p)

            # ============= Phase D: pairwise softmax attention =============
            fWv = ffW.rearrange("(k p) m -> k p m", p=128)
            rWv = frW.rearrange("(k p) m -> k p m", p=128)
            fuse_chunks = [(fWv, ffWh, k) for k in range(2 * KU)] + \
                          [(rWv, frWh, k) for k in range(2 * KU)]
            with tc.tile_pool(name="pdn", bufs=4, space="PSUM") as pdn, \
                 tc.tile_pool(name="pds", bufs=2, space="PSUM") as pds, \
                 tc.tile_pool(name="pdr", bufs=1, space="PSUM") as pdr, \
                 tc.tile_pool(name="pbc", bufs=1, space="PSUM") as pbc, \
                 tc.tile_pool(name="stgf", bufs=4) as stgf, \
                 tc.tile_pool(name="dsb", bufs=4) as dsb:
                for b in range(BPC):
                    for h in range(IH):
                        # drip-feed fuse-gate weight loads through the
                        # attention phase (gpsimd is idle here)
                        unit = b * IH + h
                        for ci in range(unit * 4, unit * 4 + 4):
                            wv_, wdst_, k_ = fuse_chunks[ci]
                            wsf = stgf.tile([128, U], F32, tag="wsf",
                                            name=f"wsf_{ci}")
                            nc.sync.dma_start(wsf, wv_[k_])
                            if ci % 2 == 0:
                                nc.vector.tensor_copy(wdst_[:, k_, :], wsf)
                            else:
                                nc.scalar.copy(wdst_[:, k_, :], wsf)
                        isl = slice(b * L + h * 512, b * L + (h + 1) * 512)
                        pn = [pdn.tile([128, 512], F32, tag="pn",
                                       name=f"pn_{b}_{h}_{du}")
                              for du in range(KU)]
                        pr = pdr.tile([1, 512], F32, tag="pr")
                        thbc = dsb.tile([128, 512], BF16, tag="thbc")
                        pb1 = pbc.tile([128, 512], F32, tag="pb",
                                       name=f"pb1_{b}_{h}")
                        nc.tensor.matmul(pb1, ones_row, thr[:, isl],
                                         start=True, stop=True)
                        nc.scalar.copy(thbc, pb1)
                        for jt in range(JT):
                            jg = b * JT + jt
                            jsl = slice(b * L + jt * 128, b * L + (jt + 1) * 128)
                            ps = pds.tile([128, 512], F32, tag="ps")
                            for k in range(KU):
                                nc.tensor.matmul(ps, w3x[:, k, jsl],
                                                 x2T[:, k, isl],
                                                 start=(k == 0),
                                                 stop=(k == KU - 1))
                            eh = dsb.tile([128, 512], BF16, tag="eh")
                            nc.scalar.activation(eh, ps, AF.Exp,
                                                 bias=s2f[:, jg:jg + 1])
                            nc.vector.tensor_tensor(eh, eh, thbc, op=OP.max)
                            for du in range(KU):
                                nc.tensor.matmul(
                                    pn[du],
                                    xO[:, jg, du * 128:(du + 1) * 128], eh,
                                    start=(jt == 0), stop=(jt == JT - 1))
                            nc.tensor.matmul(pr, ones_col, eh,
                                             start=(jt == 0),
                                             stop=(jt == JT - 1))
                        rec = dsb.tile([1, 512], F32, tag="rec")
                        nc.vector.reciprocal_approx_fast(rec, pr)
                        rech = dsb.tile([1, 512], BF16, tag="rech")
                        nc.vector.tensor_copy(rech, rec)
                        rbc = dsb.tile([128, 512], BF16, tag="rbc")
                        pb2 = pbc.tile([128, 512], F32, tag="pb",
                                       name=f"pb2_{b}_{h}")
                        nc.tensor.matmul(pb2, ones_row, rech,
                                         start=True, stop=True)
                        nc.scalar.copy(rbc, pb2)
                        # drain pn psum banks quickly via scalar, then
                        # normalize in fast bf16 on vector
                        pnh = [dsb.tile([128, 512], BF16, tag="pnh",
                                        name=f"pnh_{b}_{h}_{du}")
                               for du in range(KU)]
                        for du in range(KU):
                            if du % 2 == 0:
                                nc.scalar.copy(pnh[du], pn[du])
                            else:
                                nc.vector.tensor_copy(pnh[du], pn[du])
                        for du in range(KU):
                            nc.vector.tensor_tensor(
                                attT[:, du, isl], pnh[du], rbc, op=OP.mult)

            # ============= Phase E: fuse gates + output ====================
            with tc.tile_pool(name="pep", bufs=2, space="PSUM") as pep, \
                 tc.tile_pool(name="esb", bufs=3) as esb:
                for mt in range(NT):
                    msl = slice(mt * 128, (mt + 1) * 128)
                    x0t = esb.tile([128, U], F32, tag="x0t")
                    nc.sync.dma_start(x0t, xv[mt])
                    pz = pep.tile([128, 512], F32, tag="pz")
                    pr2 = pep.tile([128, 512], F32, tag="pr2")
                    for k in range(2 * KU):
                        lhsT = (xTh[:, k, msl] if k < KU
                                else attT[:, k - KU, msl])
                        nc.tensor.matmul(pz, lhsT, ffWh[:, k, :],
                                         start=(k == 0), stop=False)
                        nc.tensor.matmul(pr2, lhsT, frWh[:, k, :],
                                         start=(k == 0), stop=False)
                    nc.tensor.matmul(pz, ones_row, ffb_h, start=False,
                                     stop=True)
                    nc.tensor.matmul(pr2, ones_row, frb_h, start=False,
                                     stop=True)
                    zh = esb.tile([128, U], BF16, tag="zh")
                    rh = esb.tile([128, U], BF16, tag="rh")
                    nc.scalar.activation(zh, pz, AF.Sigmoid)
                    nc.scalar.activation(rh, pr2, AF.Sigmoid)
                    q = esb.tile([128, U], F32, tag="q")
                    nc.scalar.square(q, zh)
                    p2 = esb.tile([128, U], F32, tag="p2")
                    nc.vector.tensor_tensor(p2, rh, x0t, op=OP.mult)
                    ot = esb.tile([128, U], F32, tag="ot")
                    nc.vector.tensor_tensor(ot, q, p2, op=OP.add)
                    nc.sync.dma_start(outv[mt], ot)

    nc.compile()
    return nc


_NC_CACHE = None


def _get_nc():
    global _NC_CACHE
    if _NC_CACHE is None:
        _NC_CACHE = build_nc()
    return _NC_CACHE


def kernel(**inputs) -> np.ndarray:
    from concourse.bass_utils import run_bass_kernel_spmd

    nc = _get_nc()
    full = {k: np.ascontiguousarray(np.asarray(v, dtype=np.float32))
            for k, v in inputs.items()}
    in_maps = []
    for c in range(NCORES):
        m = dict(full)
        m["inputs"] = np.ascontiguousarray(
            full["inputs"][c * BPC:(c + 1) * BPC])
        in_maps.append(m)
    res = run_bass_kernel_spmd(nc, in_maps, core_ids=list(range(NCORES)))
    return np.concatenate([res.results[c]["out"] for c in range(NCORES)],
                          axis=0)


# revision 35
# speedup vs baseline: 1.0560x; 1.0050x over previous
"""Trainium2 Bass kernel for nn_Encoding_layer (highway stack + pairwise MLP
attention + fuse gates).

Sharding: data-parallel over batch B=16 across 8 NeuronCores (2 batches per
core); all dense weights replicated. No collectives.

Per-core layouts (n = 2 batches x L=1024 = 2048 token-columns):
  xTh/x1T/x2T/w3x/attT : [128, 4, 2048] bf16, "transposed" activations
                         [u mod 128, u div 128, n]
  xO                   : [128, 16, 512] bf16, row-major highway output
                         [row mod 128, row div 128, u]
  Attention: S^T[j,i] = s3[j,i] (PE, w3*x^T as lhsT) + s2[j] (ACT exp bias).
  The per-column term s1[i]+ab never enters the matmuls: exp(S+s1+ab) =
  exp(s1+ab)*exp(S), and a per-column factor cancels in the softmax, so
  relu becomes a clamp against th[i] = exp(-(s1[i]+ab)):
      M^T = max(exp(s3+s2), th)  ==  exp(relu(S_full)) / exp(s1+ab)
  Numerator att^T (lhsT = row-major x) and denominator r (lhsT = ones
  column) come from matmuls against M^T; normalization multiplies by the
  broadcast fast-approx reciprocal of r.  Broadcasts of [1,512] rows are
  PE outer-products (ones_row as lhsT) + scalar-engine copies - gpsimd
  partition_broadcast triggers multi-us ucode LIBRARY_RELOAD stalls.
"""

import numpy as np

B, L, U, H = 16, 1024, 512, 2
NCORES = 8
BPC = B // NCORES          # batches per core
N = BPC * L                # token columns per core
KU = U // 128              # 4  u-tiles
NT = N // 128              # 16 row-tiles per core
NS = N // 512              # 4  512-wide column slices per core
JT = L // 128              # 8  j-tiles per batch
IH = L // 512              # 2  i-halves per batch


def build_nc():
    import concourse.bacc as bacc
    import concourse.tile as tile
    from concourse import mybir
    from concourse.masks import make_identity

    F32 = mybir.dt.float32
    BF16 = mybir.dt.bfloat16
    AF = mybir.ActivationFunctionType
    OP = mybir.AluOpType

    nc = bacc.Bacc("TRN2", target_bir_lowering=False, debug=False,
                   num_devices=NCORES)

    x_in = nc.dram_tensor("inputs", [BPC, L, U], F32, kind="ExternalInput").ap()
    tW = nc.dram_tensor("tW", [H, U, U], F32, kind="ExternalInput").ap()
    tb = nc.dram_tensor("tb", [H, U], F32, kind="ExternalInput").ap()
    cW = nc.dram_tensor("cW", [H, U, U], F32, kind="ExternalInput").ap()
    cb = nc.dram_tensor("cb", [H, U], F32, kind="ExternalInput").ap()
    aW = nc.dram_tensor("aW", [3 * U], F32, kind="ExternalInput").ap()
    ab = nc.dram_tensor("ab", [1], F32, kind="ExternalInput").ap()
    frW = nc.dram_tensor("frW", [2 * U, U], F32, kind="ExternalInput").ap()
    frb = nc.dram_tensor("frb", [U], F32, kind="ExternalInput").ap()
    ffW = nc.dram_tensor("ffW", [2 * U, U], F32, kind="ExternalInput").ap()
    ffb = nc.dram_tensor("ffb", [U], F32, kind="ExternalInput").ap()
    out = nc.dram_tensor("out", [BPC, L, U], F32, kind="ExternalOutput").ap()

    xv = x_in.flatten_outer_dims().rearrange("(t p) u -> t p u", p=128)
    outv = out.flatten_outer_dims().rearrange("(t p) u -> t p u", p=128)

    with tile.TileContext(nc) as tc:
        with tc.tile_pool(name="pers", bufs=1) as pers:
            # ---- persistent SBUF tensors ----
            xTh = pers.tile([128, KU, N], BF16, tag="xTh")    # inputs^T
            x1T = pers.tile([128, KU, N], BF16, tag="x1T")
            x2T = pers.tile([128, KU, N], BF16, tag="x2T")
            w3x = pers.tile([128, KU, N], BF16, tag="w3x")
            attT = pers.tile([128, KU, N], BF16, tag="attT")
            xO = pers.tile([128, NT, U], BF16, tag="xO")
            tWh = pers.tile([128, H, KU, U], BF16, tag="tWh")
            cWh = pers.tile([128, H, KU, U], BF16, tag="cWh")
            ffWh = pers.tile([128, 2 * KU, U], BF16, tag="ffWh")
            frWh = pers.tile([128, 2 * KU, U], BF16, tag="frWh")
            tbsb = pers.tile([128, H, KU], F32, tag="tbsb")
            cbsb = pers.tile([128, H, KU], F32, tag="cbsb")
            awsb = pers.tile([128, 12], F32, tag="awsb")      # w1|w2|w3 cols
            w1h = pers.tile([128, KU], BF16, tag="w1h")
            w2h = pers.tile([128, KU], BF16, tag="w2h")
            ab_sb = pers.tile([1, 1], F32, tag="ab_sb")
            nab_sb = pers.tile([1, 1], F32, tag="nab_sb")
            ffb_h = pers.tile([1, U], BF16, tag="ffb_h")
            frb_h = pers.tile([1, U], BF16, tag="frb_h")
            thr = pers.tile([1, N], BF16, tag="thr")   # exp(-(s1+ab))
            s2f = pers.tile([128, NT], F32, tag="s2f")
            ones_row = pers.tile([1, 128], BF16, tag="ones_row")
            ones_col = pers.tile([128, 1], BF16, tag="ones_col")
            ident = pers.tile([128, 128], BF16, tag="ident")
            identf = pers.tile([128, 128], F32, tag="identf")

            nc.vector.memset(ones_row, 1.0)
            nc.vector.memset(ones_col, 1.0)
            make_identity(nc, ident)
            make_identity(nc, identf)

            # ================= Phase A: loads, casts, input transpose ======
            with tc.tile_pool(name="stg", bufs=8) as stg, \
                 tc.tile_pool(name="stgw", bufs=8) as stgw, \
                 tc.tile_pool(name="ptA", bufs=1, space="PSUM") as ptA:
                # inputs^T via PE transpose (PE is idle here), with
                # highway-weight loads interleaved after tg0/tg1 so layer-0
                # can start as soon as the first column group lands
                def emit_weights(l, wi):
                    wsrc, wdst = ((tW, tWh), (cW, cWh))[wi]
                    wv = wsrc[l].rearrange("(k p) m -> k p m", p=128)
                    for k in range(KU):
                        ws = stgw.tile([128, U], F32, tag="ws",
                                       name=f"ws_{l}_{wi}_{k}")
                        nc.sync.dma_start(ws, wv[k])
                        if k % 2 == 0:
                            nc.vector.tensor_copy(wdst[:, l, k, :], ws)
                        else:
                            nc.scalar.copy(wdst[:, l, k, :], ws)

                # warm the PE HAM clock-gate during the initial DMA wait:
                # ~40 tiny matmuls lift the PE to 2.4 GHz before the fp32
                # transposes (which never count as HAM-busy) begin
                warm = [ptA.tile([128, 512], F32, tag=f"ptk{k}",
                                 name=f"warm_{k}") for k in range(KU)]
                for i in range(56):
                    nc.tensor.matmul(warm[i % KU][:, 0:128], ident, ident,
                                     start=True, stop=True)
                for tg in range(NS):
                    ptk = [ptA.tile([128, 512], F32, tag=f"ptk{k}",
                                    name=f"ptk_{tg}_{k}")
                           for k in range(KU)]
                    for tt in range(4):
                        t = tg * 4 + tt
                        xs = stg.tile([128, U], F32, tag="xs",
                                      name=f"xs_{t}")
                        nc.sync.dma_start(xs, xv[t])
                        for k in range(KU):
                            nc.tensor.transpose(
                                ptk[k][:, tt * 128:(tt + 1) * 128],
                                xs[:, k * 128:(k + 1) * 128], identf)
                    for k in range(KU):
                        if k % 2 == 0:
                            nc.vector.tensor_copy(
                                xTh[:, k, tg * 512:(tg + 1) * 512], ptk[k])
                        else:
                            nc.scalar.copy(
                                xTh[:, k, tg * 512:(tg + 1) * 512], ptk[k])
                    if tg < H:
                        emit_weights(0, tg)
                    elif tg == H:
                        nc.sync.dma_start(
                            tbsb, tb.rearrange("l (m p) -> p l m", p=128))
                        nc.sync.dma_start(
                            cbsb, cb.rearrange("l (m p) -> p l m", p=128))
                        nc.sync.dma_start(
                            awsb, aW.rearrange("(w m p) -> p (w m)",
                                               p=128, w=3))
                        nc.vector.tensor_copy(w1h, awsb[:, 0:KU])
                        nc.vector.tensor_copy(w2h, awsb[:, KU:2 * KU])
                        nc.sync.dma_start(ab_sb, ab[None, :])
                        nc.scalar.mul(nab_sb, ab_sb, -1.0)
                        fb = stg.tile([1, U], F32, tag="fb")
                        nc.sync.dma_start(fb, ffb[None, :])
                        nc.vector.tensor_copy(ffb_h, fb)
                        fb2 = stg.tile([1, U], F32, tag="fb")
                        nc.sync.dma_start(fb2, frb[None, :])
                        nc.vector.tensor_copy(frb_h, fb2)
                    else:
                        emit_weights(1, 0)
                        emit_weights(1, 1)

            # ============= Phase B: highway stack (2 layers) ===========
            with tc.tile_pool(name="hwp", bufs=2, space="PSUM") as hwp, \
                 tc.tile_pool(name="hws", bufs=3) as hws:
                for l in range(H):
                    xin = xTh if l == 0 else x1T
                    xout = x1T if l == 0 else x2T
                    for t in range(NS):
                        nsl = slice(t * 512, (t + 1) * 512)
                        for m in range(KU):
                            pt = hwp.tile([128, 512], F32, tag="pt")
                            pc = hwp.tile([128, 512], F32, tag="pc")
                            for k in range(KU):
                                nc.tensor.matmul(
                                    pt, tWh[:, l, k, m * 128:(m + 1) * 128],
                                    xin[:, k, nsl],
                                    start=(k == 0), stop=(k == KU - 1))
                            for k in range(KU):
                                nc.tensor.matmul(
                                    pc, cWh[:, l, k, m * 128:(m + 1) * 128],
                                    xin[:, k, nsl],
                                    start=(k == 0), stop=(k == KU - 1))
                            th = hws.tile([128, 512], BF16, tag="th")
                            ch = hws.tile([128, 512], BF16, tag="ch")
                            nc.scalar.activation(
                                th, pt, AF.Relu, bias=tbsb[:, l, m:m + 1])
                            nc.scalar.activation(
                                ch, pc, AF.Sigmoid, bias=cbsb[:, l, m:m + 1])
                            dh = hws.tile([128, 512], BF16, tag="dh")
                            nc.vector.tensor_tensor(
                                dh, th, xin[:, m, nsl], op=OP.subtract)
                            mh = hws.tile([128, 512], BF16, tag="mh")
                            nc.vector.tensor_tensor(
                                mh, ch, dh, op=OP.mult)
                            nc.gpsimd.tensor_tensor(
                                xout[:, m, nsl], xin[:, m, nsl], mh,
                                op=OP.add)

            # ============= Phase C: attention prep =========================
            with tc.tile_pool(name="pcp", bufs=2, space="PSUM") as pcp, \
                 tc.tile_pool(name="pcp1", bufs=1, space="PSUM") as pcp1:
                # w3 * x^T  (w3 is per-partition here)
                for k in range(KU):
                    nc.vector.tensor_scalar_mul(
                        w3x[:, k, :], x2T[:, k, :], awsb[:, 8 + k:9 + k])
                # x back to row-major via PE transpose (bf16, psum staging)
                for jt in range(NT):
                    ptr = pcp.tile([128, 512], BF16, tag="ptr")
                    for k in range(KU):
                        nc.tensor.transpose(
                            ptr[:, k * 128:(k + 1) * 128],
                            x2T[:, k, jt * 128:(jt + 1) * 128], ident)
                    nc.vector.tensor_copy(xO[:, jt, :], ptr)
                # clamp threshold exp(-(s1+ab)); the factor exp(s1+ab)
                # cancels in the softmax so it never enters the matmuls
                for t in range(NS):
                    ps1 = pcp1.tile([1, 512], F32, tag="ps1")
                    for k in range(KU):
                        nc.tensor.matmul(ps1, w1h[:, k:k + 1],
                                         x2T[:, k, t * 512:(t + 1) * 512],
                                         start=(k == 0), stop=(k == KU - 1))
                    nc.scalar.activation(
                        thr[:, t * 512:(t + 1) * 512], ps1, AF.Exp,
                        bias=nab_sb, scale=-1.0)
                # s2 = x @ w2: all 16 j-tiles into one psum bank, one copy
                s2p = pcp1.tile([128, NT], F32, tag="s2p")
                for jt in range(NT):
                    for k in range(KU):
                        nc.tensor.matmul(s2p[:, jt:jt + 1],
                                         x2T[:, k, jt * 128:(jt + 1) * 128],
                                         w2h[:, k:k + 1],
                                         start=(k == 0), stop=(k == KU - 1))
                nc.vector.tensor_copy(s2f, s2p)

            # ============= Phase D: pairwise softmax attention =============
            fWv = ffW.rearrange("(k p) m -> k p m", p=128)
            rWv = frW.rearrange("(k p) m -> k p m", p=128)
            fuse_chunks = [(fWv, ffWh, k) for k in range(2 * KU)] + \
                          [(rWv, frWh, k) for k in range(2 * KU)]
            with tc.tile_pool(name="pdn", bufs=4, space="PSUM") as pdn, \
                 tc.tile_pool(name="pds", bufs=2, space="PSUM") as pds, \
                 tc.tile_pool(name="pdr", bufs=1, space="PSUM") as pdr, \
                 tc.tile_pool(name="pbc", bufs=1, space="PSUM") as pbc, \
                 tc.tile_pool(name="stgf", bufs=4) as stgf, \
                 tc.tile_pool(name="dsb", bufs=4) as dsb:
                for b in range(BPC):
                    for h in range(IH):
                        # drip-feed fuse-gate weight loads through the
                        # attention phase (gpsimd is idle here)
                        unit = b * IH + h
                        for ci in range(unit * 4, unit * 4 + 4):
                            wv_, wdst_, k_ = fuse_chunks[ci]
                            wsf = stgf.tile([128, U], F32, tag="wsf",
                                            name=f"wsf_{ci}")
                            nc.sync.dma_start(wsf, wv_[k_])
                            if ci % 2 == 0:
                                nc.vector.tensor_copy(wdst_[:, k_, :], wsf)
                            else:
                                nc.scalar.copy(wdst_[:, k_, :], wsf)
                        isl = slice(b * L + h * 512, b * L + (h + 1) * 512)
                        pn = [pdn.tile([128, 512], F32, tag="pn",
                                       name=f"pn_{b}_{h}_{du}")
                              for du in range(KU)]
                        pr = pdr.tile([1, 512], F32, tag="pr")
                        thbc = dsb.tile([128, 512], BF16, tag="thbc")
                        pb1 = pbc.tile([128, 512], F32, tag="pb",
                                       name=f"pb1_{b}_{h}")
                        nc.tensor.matmul(pb1, ones_row, thr[:, isl],
                                         start=True, stop=True)
                        nc.scalar.copy(thbc, pb1)
                        for jt in range(JT):
                            jg = b * JT + jt
                            jsl = slice(b * L + jt * 128, b * L + (jt + 1) * 128)
                            ps = pds.tile([128, 512], F32, tag="ps")
                            for k in range(KU):
                                nc.tensor.matmul(ps, w3x[:, k, jsl],
                                                 x2T[:, k, isl],
                                                 start=(k == 0),
                                                 stop=(k == KU - 1))
                            eh = dsb.tile([128, 512], BF16, tag="eh")
                            nc.scalar.activation(eh, ps, AF.Exp,
                                                 bias=s2f[:, jg:jg + 1])
                            nc.vector.tensor_tensor(eh, eh, thbc, op=OP.max)
                            for du in range(KU):
                                nc.tensor.matmul(
                                    pn[du],
                                    xO[:, jg, du * 128:(du + 1) * 128], eh,
                                    start=(jt == 0), stop=(jt == JT - 1))
                            nc.tensor.matmul(pr, ones_col, eh,
                                             start=(jt == 0),
                                             stop=(jt == JT - 1))
                        rec = dsb.tile([1, 512], F32, tag="rec")
                        nc.vector.reciprocal_approx_fast(rec, pr)
                        rech = dsb.tile([1, 512], BF16, tag="rech")
                        nc.vector.tensor_copy(rech, rec)
                        rbc = dsb.tile([128, 512], BF16, tag="rbc")
                        pb2 = pbc.tile([128, 512], F32, tag="pb",
                                       name=f"pb2_{b}_{h}")
                        nc.tensor.matmul(pb2, ones_row, rech,
                                         start=True, stop=True)
                        nc.scalar.copy(rbc, pb2)
                        # drain pn psum banks quickly via scalar, then
                        # normalize in fast bf16 on vector
                        pnh = [dsb.tile([128, 512], BF16, tag="pnh",
                                        name=f"pnh_{b}_{h}_{du}")
                               for du in range(KU)]
                        for du in range(KU):
                            if du % 2 == 0:
                                nc.scalar.copy(pnh[du], pn[du])
                            else:
                                nc.vector.tensor_copy(pnh[du], pn[du])
                        for du in range(KU):
                            nc.vector.tensor_tensor(
                                attT[:, du, isl], pnh[du], rbc, op=OP.mult)

            # ============= Phase E: fuse gates + output ====================
            with tc.tile_pool(name="pep", bufs=2, space="PSUM") as pep, \
                 tc.tile_pool(name="esb", bufs=3) as esb:
                for mt in range(NT):
                    msl = slice(mt * 128, (mt + 1) * 128)
                    x0t = esb.tile([128, U], F32, tag="x0t")
                    nc.sync.dma_start(x0t, xv[mt])
                    pz = pep.tile([128, 512], F32, tag="pz")
                    pr2 = pep.tile([128, 512], F32, tag="pr2")
                    for k in range(2 * KU):
                        lhsT = (xTh[:, k, msl] if k < KU
                                else attT[:, k - KU, msl])
                        nc.tensor.matmul(pz, lhsT, ffWh[:, k, :],
                                         start=(k == 0), stop=False)
                        nc.tensor.matmul(pr2, lhsT, frWh[:, k, :],
                                         start=(k == 0), stop=False)
                    nc.tensor.matmul(pz, ones_row, ffb_h, start=False,
                                     stop=True)
                    nc.tensor.matmul(pr2, ones_row, frb_h, start=False,
                                     stop=True)
                    zh = esb.tile([128, U], BF16, tag="zh")
                    rh = esb.tile([128, U], BF16, tag="rh")
                    q = esb.tile([128, U], F32, tag="q")
                    p2 = esb.tile([128, U], F32, tag="p2")
                    ot = esb.tile([128, U], F32, tag="ot")
                    if mt == NT - 1:
                        # last unit sets the kernel tail: shorten its
                        # serial chain by splitting across engines
                        hU = U // 2
                        nc.scalar.activation(zh, pz, AF.Sigmoid)
                        nc.scalar.square(q, zh)
                        nc.scalar.activation(rh, pr2, AF.Sigmoid)
                        nc.vector.tensor_tensor(p2[:, :hU], rh[:, :hU],
                                                x0t[:, :hU], op=OP.mult)
                        nc.gpsimd.tensor_tensor(p2[:, hU:], rh[:, hU:],
                                                x0t[:, hU:], op=OP.mult)
                        nc.vector.tensor_tensor(ot[:, :hU], q[:, :hU],
                                                p2[:, :hU], op=OP.add)
                        nc.gpsimd.tensor_tensor(ot[:, hU:], q[:, hU:],
                                                p2[:, hU:], op=OP.add)
                    else:
                        nc.scalar.activation(zh, pz, AF.Sigmoid)
                        nc.scalar.activation(rh, pr2, AF.Sigmoid)
                        nc.scalar.square(q, zh)
                        nc.vector.tensor_tensor(p2, rh, x0t, op=OP.mult)
                        nc.vector.tensor_tensor(ot, q, p2, op=OP.add)
                    nc.sync.dma_start(outv[mt], ot)

    nc.compile()
    return nc


_NC_CACHE = None


def _get_nc():
    global _NC_CACHE
    if _NC_CACHE is None:
        _NC_CACHE = build_nc()
    return _NC_CACHE


def kernel(**inputs) -> np.ndarray:
    from concourse.bass_utils import run_bass_kernel_spmd

    nc = _get_nc()
    full = {k: np.ascontiguousarray(np.asarray(v, dtype=np.float32))
            for k, v in inputs.items()}
    in_maps = []
    for c in range(NCORES):
        m = dict(full)
        m["inputs"] = np.ascontiguousarray(
            full["inputs"][c * BPC:(c + 1) * BPC])
        in_maps.append(m)
    res = run_bass_kernel_spmd(nc, in_maps, core_ids=list(range(NCORES)))
    return np.concatenate([res.results[c]["out"] for c in range(NCORES)],
                          axis=0)
